# revision 9
# baseline (speedup 1.0000x reference)
"""Expert-parallel MoE MLP kernel for Trainium2 (8 NeuronCores).

Problem: out[b,e,n,d] = gelu(x[b,e] @ w1[e] + b1[e]) @ w2[e] + b2[e]
Shapes: x [2,8,1024,1024] f32, w1 [8,1024,4096], b1 [8,4096],
        w2 [8,4096,1024], b2 [8,1024].

Sharding: expert e -> core e. Each core runs a 2048-token MLP:
  [2048,1024] @ [1024,4096] -> gelu -> @ [4096,1024] -> [2048,1024]

Device-side layout: activations live transposed ([feature, token]) so the
contraction dim is always the SBUF partition dim:
  phase 1: psum[h_tile, t] += w1[d_tile, h_tile].T @ xT[d_tile, t]
  phase 2: psum[d_tile, t] += w2[h_tile, d_tile].T @ hT[h_tile, t]
Host transposes x on the way in and out on the way back (part of
shard/unshard), so the device does zero transposes.

All matmul inputs are bf16 (fp32 PSUM accumulation); GELU (tanh approx,
matching jax.nn.gelu default) fused with the b1 add on ScalarE.

fast_io (default): w1 is host-pretiled to [nh, P, kd*P] and biases
host-transposed to [P, .] so every weight/bias DMA reads contiguous
per-partition lines (2KB bursts instead of scattered 256B/4B), and the
biases issue on the gpsimd queue so the SP queue starts streaming w1
immediately after the prologue. NTFF-measured 468-471us vs the 437us
bf16 PE roofline; the stream itself runs at the 215ns/matmul
back-to-back floor with LDWEIGHTS fully hidden, the remainder being
fixed prologue/teardown and the initial x/w DMA ramp.
"""

import sys

for _p in ("/opt/trn_rl_repo", "/root/.axon_site/_ro/trn_rl_repo"):
    if _p not in sys.path:
        sys.path.insert(0, _p)

import numpy as np
import ml_dtypes

from contextlib import ExitStack

import concourse.bass as bass
import concourse.tile as tile
from concourse import bacc, mybir
from concourse.bass import _add_dep_helper
from concourse.bass_utils import run_bass_kernel_spmd

BF16 = mybir.dt.bfloat16
F32 = mybir.dt.float32

# Full-problem constants (hardcoded per harness contract).
B, E, N, D, H = 2, 8, 1024, 1024, 4096
T = B * N          # tokens per expert/core
TBLK = 512         # tokens per block (= one PSUM bank of fp32)
P = 128


def build_nc(t=T, d=D, h=H, tblk=TBLK, act=None, repeats=1,
             ps_bufs=2, act_mode="gelu", phases=(1, 2), x_mode="stream",
             chain_pe=False, out_q="scalar", fast_io=False,
             x_q="gpsimd", bias_q=None, blocks=None, w2_q="sync",
             x0_chunked=False):
    """Build the per-core Bass program. All cores run this same program on
    different data (SPMD). repeats>1 re-runs the token-block loop (weights
    stay resident) — used only for steady-state timing measurements.
    act_mode: "gelu" | "copy_dve" (diagnostic: replace gelu w/ DVE copy)."""
    if act is None:
        act = mybir.ActivationFunctionType.Gelu_apprx_tanh
    kd = d // P        # contraction tiles for phase 1
    nh = h // P        # h tiles (phase-1 outputs / phase-2 contraction)
    nd = d // P        # d tiles (phase-2 outputs)
    nblk = t // tblk

    nc = bacc.Bacc("TRN2", target_bir_lowering=False)

    xt_hbm = nc.dram_tensor("xt", [d, t], BF16, kind="ExternalInput").ap()
    if fast_io:
        # w1 host-pretiled: [nh, P, kd*P]; per-partition lines contiguous
        # (2KB DMA bursts instead of scattered 256B).
        w1_hbm = nc.dram_tensor("w1", [nh, P, kd * P], BF16,
                                kind="ExternalInput").ap()
        # biases host-pre-transposed: [P, nh] / [P, nd]
        b1_hbm = nc.dram_tensor("b1", [P, nh], F32, kind="ExternalInput").ap()
        b2_hbm = nc.dram_tensor("b2", [P, nd], F32, kind="ExternalInput").ap()
    else:
        w1_hbm = nc.dram_tensor("w1", [d, h], BF16, kind="ExternalInput").ap()
        b1_hbm = nc.dram_tensor("b1", [nh, P], F32, kind="ExternalInput").ap()
        b2_hbm = nc.dram_tensor("b2", [nd, P], F32, kind="ExternalInput").ap()
    w2_hbm = nc.dram_tensor("w2", [h, d], BF16, kind="ExternalInput").ap()
    out_hbm = nc.dram_tensor("outT", [d, t], F32, kind="ExternalOutput").ap()

    # [feature, x] views with the 128-partition dim innermost in features.
    xt_v = xt_hbm.rearrange("(kd p) t -> p kd t", p=P)
    if fast_io:
        w1_v = w1_hbm.rearrange("ih p k -> p ih k")
    else:
        w1_v = w1_hbm.rearrange("(kd p) h -> p kd h", p=P)
    w2_v = w2_hbm.rearrange("(kh p) d -> p kh d", p=P)

    with tile.TileContext(nc) as tc, ExitStack() as ctx:
        w1_pool = ctx.enter_context(tc.tile_pool(name="w1", bufs=nh))
        w2_pool = ctx.enter_context(tc.tile_pool(name="w2", bufs=nh))
        x_pool = ctx.enter_context(tc.tile_pool(name="x", bufs=2))
        h_pool = ctx.enter_context(tc.tile_pool(name="h", bufs=nh + 2))
        o_pool = ctx.enter_context(tc.tile_pool(name="o", bufs=4))
        c_pool = ctx.enter_context(tc.tile_pool(name="c", bufs=1))
        ps1 = ctx.enter_context(tc.tile_pool(name="ps1", bufs=ps_bufs, space="PSUM"))
        ps2 = ctx.enter_context(tc.tile_pool(name="ps2", bufs=ps_bufs, space="PSUM"))

        # Biases, resident. fast_io: already [P, x] on host, issued on the
        # gpsimd queue so the sync queue starts weight chunks immediately.
        b1_sb = c_pool.tile([P, nh], F32)
        b2_sb = c_pool.tile([P, nd], F32)
        if fast_io:
            bq = getattr(nc, bias_q or "gpsimd")
            bq.dma_start(out=b1_sb, in_=b1_hbm)
            bq.dma_start(out=b2_sb, in_=b2_hbm)
        else:
            nc.sync.dma_start(out=b1_sb, in_=b1_hbm.rearrange("t p -> p t"))
            nc.sync.dma_start(out=b2_sb, in_=b2_hbm.rearrange("t p -> p t"))

        if blocks is None:
            blk_sizes = [tblk] * (t // tblk)
        else:
            assert sum(blocks) == t, (blocks, t)
            blk_sizes = list(blocks)
        blk_off = [sum(blk_sizes[:i]) for i in range(len(blk_sizes))]
        nb = len(blk_sizes)
        xdma = getattr(nc, x_q)

        # Block-0 x issued first (per-ik chunks) so the first chain can
        # start as soon as chunk 0 + w1[0] land.
        x0_parts = None
        if x0_chunked and x_mode == "stream":
            x0_parts = []
            tb0 = blk_sizes[0]
            for ik in range(kd):
                xc = x_pool.tile([P, tb0], BF16, name=f"x0c{ik}",
                                 tag=f"x0c{ik}")
                xdma.dma_start(out=xc, in_=xt_v[:, ik, 0:tb0])
                x0_parts.append(xc)

        # Weights, resident in SBUF for the whole kernel. Chunked DMAs so
        # compute can start as soon as the first chunks land.
        w1_t = []
        for ih in range(nh):
            wt = w1_pool.tile([P, kd, P], BF16)
            if fast_io:
                nc.sync.dma_start(
                    out=wt.rearrange("p a b -> p (a b)"), in_=w1_v[:, ih, :])
            else:
                nc.sync.dma_start(out=wt, in_=w1_v[:, :, ih * P:(ih + 1) * P])
            w1_t.append(wt)
        w2_t = []
        for ikh in range(nh):
            wt = w2_pool.tile([P, d], BF16)
            getattr(nc, w2_q).dma_start(out=wt, in_=w2_v[:, ikh, :])
            w2_t.append(wt)

        prev_mm = [None]

        def MM(*args, **kwargs):
            bi = nc.tensor.matmul(*args, **kwargs)
            if chain_pe and prev_mm[0] is not None:
                _add_dep_helper(bi.ins, prev_mm[0].ins, sync=False,
                                reason="pe emission order")
            prev_mm[0] = bi
            return bi

        gelu = act
        xt_pre = {}
        if x_mode == "preload":
            for ib in range(nblk):
                xt_pre[ib] = c_pool.tile([P, kd, tblk], BF16,
                                         name=f"xp{ib}", tag=f"xp{ib}")
                nc.sync.dma_start(
                    out=xt_pre[ib],
                    in_=xt_v[:, :, ib * tblk:(ib + 1) * tblk])
        for ib in [i % nb for i in range(nb * repeats)]:
            tb = blk_sizes[ib]
            tsl = slice(blk_off[ib], blk_off[ib] + tb)
            if x0_parts is not None and ib == 0:
                xts = [x0_parts[ik] for ik in range(kd)]
            elif x_mode == "preload":
                xt = xt_pre[ib]
                xts = [xt[:, ik, :] for ik in range(kd)]
            else:
                xt = x_pool.tile([P, kd, tb], BF16, name=f"xt{tb}",
                                 tag=f"xt{tb}")
                xdma.dma_start(out=xt, in_=xt_v[:, :, tsl])
                xts = [xt[:, ik, :] for ik in range(kd)]

            # phase 1: hT[h_tile] = gelu(w1.T @ xT + b1)
            ht = []
            if 1 in phases:
                for ih in range(nh):
                    ps = ps1.tile([P, tblk], F32)
                    for ik in range(kd):
                        MM(
                            ps[:, :tb], w1_t[ih][:, ik, :], xts[ik],
                            start=(ik == 0), stop=(ik == kd - 1),
                        )
                    hs = h_pool.tile([P, tb], BF16, name=f"hs{tb}",
                                     tag=f"hs{tb}")
                    if act_mode == "gelu":
                        nc.scalar.activation(hs, ps[:, :tb], gelu,
                                             bias=b1_sb[:, ih:ih + 1])
                    else:
                        nc.vector.tensor_copy(hs, ps[:, :tb])
                    ht.append(hs)
            else:
                # diagnostic: fake hT from xt slices (kd divides nh usage)
                for ih in range(nh):
                    hs = h_pool.tile([P, tb], BF16, name=f"hs{tb}",
                                     tag=f"hs{tb}")
                    nc.vector.tensor_copy(hs, xts[ih % kd])
                    ht.append(hs)

            # phase 2: outT[d_tile] = w2.T @ hT + b2
            if 2 in phases:
                for idt in range(nd):
                    ps = ps2.tile([P, tblk], F32)
                    for ikh in range(nh):
                        MM(
                            ps[:, :tb], w2_t[ikh][:, idt * P:(idt + 1) * P],
                            ht[ikh],
                            start=(ikh == 0), stop=(ikh == nh - 1),
                        )
                    ob = o_pool.tile([P, tb], F32, name=f"ob{tb}",
                                     tag=f"ob{tb}")
                    nc.vector.tensor_scalar_add(ob, ps[:, :tb],
                                                b2_sb[:, idt:idt + 1])
                    getattr(nc, out_q).dma_start(
                        out=out_hbm[idt * P:(idt + 1) * P, tsl], in_=ob
                    )
            elif 1 in phases:
                # keep outputs observable so phase-1 work isn't dead
                idt = 0
                ob = o_pool.tile([P, tb], F32, name=f"ob{tb}", tag=f"ob{tb}")
                nc.vector.tensor_copy(ob, ht[ib % nh])
                nc.scalar.dma_start(
                    out=out_hbm[idt * P:(idt + 1) * P, tsl], in_=ob
                )

    nc.compile()
    return nc


# Config used by kernel() — the tuned fast-IO layout.
DEFAULT_CFG = dict(fast_io=True)

_NC_CACHE = {}


def _get_nc():
    if "nc" not in _NC_CACHE:
        _NC_CACHE["nc"] = build_nc(**DEFAULT_CFG)
    return _NC_CACHE["nc"]


def pack_w1(w1e, d=D, h=H):
    """Host pre-tiling of one expert's w1 [d,h] -> [nh, P, kd*P] bf16 so
    each per-ih DMA reads contiguous 2KB per partition."""
    kd, nh = d // P, h // P
    bf16 = ml_dtypes.bfloat16
    w = np.asarray(w1e, np.float32).astype(bf16)
    return np.ascontiguousarray(
        w.reshape(kd, P, nh, P).transpose(2, 1, 0, 3).reshape(nh, P, kd * P))


def _make_in_maps(x, w1, b1, w2, b2):
    bf16 = ml_dtypes.bfloat16
    nh, nd = H // P, D // P
    in_maps = []
    for e in range(E):
        xe = np.asarray(x[:, e], dtype=np.float32).reshape(T, D)
        in_maps.append({
            "xt": np.ascontiguousarray(xe.T).astype(bf16),
            "w1": pack_w1(w1[e]),
            "w2": np.asarray(w2[e], dtype=np.float32).astype(bf16),
            "b1": np.ascontiguousarray(
                np.asarray(b1[e], np.float32).reshape(nh, P).T),
            "b2": np.ascontiguousarray(
                np.asarray(b2[e], np.float32).reshape(nd, P).T),
        })
    return in_maps


def kernel(x, w1, b1, w2, b2):
    nc = _get_nc()
    in_maps = _make_in_maps(x, w1, b1, w2, b2)
    res = run_bass_kernel_spmd(nc, in_maps, core_ids=list(range(E)))

    out = np.empty((B, E, N, D), dtype=np.float32)
    for e in range(E):
        ot = np.asarray(res.results[e]["outT"])            # [D, T]
        out[:, e] = ot.T.reshape(B, N, D)
    return out



# revision 15
# speedup vs baseline: 1.0061x; 1.0061x over previous
"""Expert-parallel MoE MLP kernel for Trainium2 (8 NeuronCores).

Problem: out[b,e,n,d] = gelu(x[b,e] @ w1[e] + b1[e]) @ w2[e] + b2[e]
Shapes: x [2,8,1024,1024] f32, w1 [8,1024,4096], b1 [8,4096],
        w2 [8,4096,1024], b2 [8,1024].

Sharding: expert e -> core e. Each core runs a 2048-token MLP:
  [2048,1024] @ [1024,4096] -> gelu -> @ [4096,1024] -> [2048,1024]

Device-side layout: activations live transposed ([feature, token]) so the
contraction dim is always the SBUF partition dim:
  phase 1: psum[h_tile, t] += w1[d_tile, h_tile].T @ xT[d_tile, t]
  phase 2: psum[d_tile, t] += w2[h_tile, d_tile].T @ hT[h_tile, t]
Host transposes x on the way in and out on the way back (part of
shard/unshard), so the device does zero transposes.

All matmul inputs are bf16 (fp32 PSUM accumulation); GELU (tanh approx,
matching jax.nn.gelu default) fused with the b1 add on ScalarE.

fast_io (default): w1 is host-pretiled to [nh, P, kd*P] and biases
host-transposed to [P, .] so every weight/bias DMA reads contiguous
per-partition lines (2KB bursts instead of scattered 256B/4B); biases
issue on the gpsimd queue behind block-0 x so the SP queue streams w1
immediately after the prologue. x0_chunked splits block-0 x into
per-k-tile DMAs so the first chain trickles as chunks land instead of
blocking on the full 1MB transfer. NTFF-measured 468-470us vs the
437us bf16 PE roofline; the matmul stream runs at the 215ns/matmul
back-to-back floor with LDWEIGHTS fully hidden under the moving
stream, the remainder being the fixed Bacc/Tile prologue+teardown
(~14us of barriers and semaphore clears) and the HBM-bound initial
x/w1 DMA ramp (~12us). Measured dead ends kept as options: Ldweights
dedup + phase-separated weight reuse (psum-bank cycling costs more),
block taper (sub-512 matmuls expose LDWEIGHTS), x0 on the SP queue
(delays the w1 stream whose completions gate the chains).
"""

import sys

for _p in ("/opt/trn_rl_repo", "/root/.axon_site/_ro/trn_rl_repo"):
    if _p not in sys.path:
        sys.path.insert(0, _p)

import numpy as np
import ml_dtypes

from contextlib import ExitStack

import concourse.bass as bass
import concourse.tile as tile
from concourse import bacc, mybir
from concourse.bass import _add_dep_helper
from concourse.bass_utils import run_bass_kernel_spmd

BF16 = mybir.dt.bfloat16
F32 = mybir.dt.float32

# Full-problem constants (hardcoded per harness contract).
B, E, N, D, H = 2, 8, 1024, 1024, 4096
T = B * N          # tokens per expert/core
TBLK = 512         # tokens per block (= one PSUM bank of fp32)
P = 128


def build_nc(t=T, d=D, h=H, tblk=TBLK, act=None, repeats=1,
             ps_bufs=2, act_mode="gelu", phases=(1, 2), x_mode="stream",
             chain_pe=False, out_q="scalar", fast_io=False,
             x_q="gpsimd", bias_q=None, blocks=None, w2_q="sync",
             x0_chunked=False, x0_q=None, x0_manual=False):
    """Build the per-core Bass program. All cores run this same program on
    different data (SPMD). repeats>1 re-runs the token-block loop (weights
    stay resident) — used only for steady-state timing measurements.
    act_mode: "gelu" | "copy_dve" (diagnostic: replace gelu w/ DVE copy)."""
    if act is None:
        act = mybir.ActivationFunctionType.Gelu_apprx_tanh
    kd = d // P        # contraction tiles for phase 1
    nh = h // P        # h tiles (phase-1 outputs / phase-2 contraction)
    nd = d // P        # d tiles (phase-2 outputs)
    nblk = t // tblk

    nc = bacc.Bacc("TRN2", target_bir_lowering=False)

    xt_hbm = nc.dram_tensor("xt", [d, t], BF16, kind="ExternalInput").ap()
    if fast_io:
        # w1 host-pretiled: [nh, P, kd*P]; per-partition lines contiguous
        # (2KB DMA bursts instead of scattered 256B).
        w1_hbm = nc.dram_tensor("w1", [nh, P, kd * P], BF16,
                                kind="ExternalInput").ap()
        # biases host-pre-transposed: [P, nh] / [P, nd]
        b1_hbm = nc.dram_tensor("b1", [P, nh], F32, kind="ExternalInput").ap()
        b2_hbm = nc.dram_tensor("b2", [P, nd], F32, kind="ExternalInput").ap()
    else:
        w1_hbm = nc.dram_tensor("w1", [d, h], BF16, kind="ExternalInput").ap()
        b1_hbm = nc.dram_tensor("b1", [nh, P], F32, kind="ExternalInput").ap()
        b2_hbm = nc.dram_tensor("b2", [nd, P], F32, kind="ExternalInput").ap()
    w2_hbm = nc.dram_tensor("w2", [h, d], BF16, kind="ExternalInput").ap()
    out_hbm = nc.dram_tensor("outT", [d, t], F32, kind="ExternalOutput").ap()

    # [feature, x] views with the 128-partition dim innermost in features.
    xt_v = xt_hbm.rearrange("(kd p) t -> p kd t", p=P)
    if fast_io:
        w1_v = w1_hbm.rearrange("ih p k -> p ih k")
    else:
        w1_v = w1_hbm.rearrange("(kd p) h -> p kd h", p=P)
    w2_v = w2_hbm.rearrange("(kh p) d -> p kh d", p=P)

    x0_manual_parts = None
    x0_manual_dmas = None
    if x0_manual:
        # Block-0 x staged into manually-allocated SBUF, DMAd on the sync
        # queue BEFORE the TileContext entry barrier. Safety: the first
        # matmul waits on the (tile-managed) w1[0] chunk DMA semaphore,
        # which is queued on the same sync queue AFTER these transfers —
        # FIFO completion order makes the x0 data transitively ready.
        x0_manual_parts = []
        x0_manual_dmas = []
        tb0 = (blocks[0] if blocks is not None else tblk)
        x0sem = nc.semaphore("x0sem").__enter__()
        for ik in range(kd):
            xm = nc.alloc_sbuf_tensor(f"x0m{ik}", [P, tb0], BF16).ap()
            dma = nc.sync.dma_start(out=xm, in_=xt_v[:, ik, 0:tb0])
            dma.then_inc(x0sem, 16)
            x0_manual_parts.append(xm)
            x0_manual_dmas.append(dma)

    with tile.TileContext(nc) as tc, ExitStack() as ctx:
        w1_pool = ctx.enter_context(tc.tile_pool(name="w1", bufs=nh))
        w2_pool = ctx.enter_context(tc.tile_pool(name="w2", bufs=nh))
        x_pool = ctx.enter_context(tc.tile_pool(name="x", bufs=2))
        h_pool = ctx.enter_context(tc.tile_pool(name="h", bufs=nh + 2))
        o_pool = ctx.enter_context(tc.tile_pool(name="o", bufs=4))
        c_pool = ctx.enter_context(tc.tile_pool(name="c", bufs=1))
        ps1 = ctx.enter_context(tc.tile_pool(name="ps1", bufs=ps_bufs, space="PSUM"))
        ps2 = ctx.enter_context(tc.tile_pool(name="ps2", bufs=ps_bufs, space="PSUM"))

        if blocks is None:
            blk_sizes = [tblk] * (t // tblk)
        else:
            assert sum(blocks) == t, (blocks, t)
            blk_sizes = list(blocks)
        blk_off = [sum(blk_sizes[:i]) for i in range(len(blk_sizes))]
        nb = len(blk_sizes)
        xdma = getattr(nc, x_q)

        # Block-0 x issued first (per-ik chunks) so the first chain can
        # start as soon as chunk 0 + w1[0] land.
        x0_parts = x0_manual_parts
        if x0_parts is None and x0_chunked and x_mode == "stream":
            x0_pool = ctx.enter_context(tc.tile_pool(name="x0", bufs=1))
            x0_parts = []
            tb0 = blk_sizes[0]
            x0dma = getattr(nc, x0_q) if x0_q else xdma
            for ik in range(kd):
                xc = x0_pool.tile([P, tb0], BF16, name=f"x0c{ik}",
                                  tag=f"x0c{ik}")
                x0dma.dma_start(out=xc, in_=xt_v[:, ik, 0:tb0])
                x0_parts.append(xc)

        # Biases, resident. fast_io: already [P, x] on host, issued on the
        # gpsimd queue AFTER block-0 x so x0 transfers gate nothing.
        b1_sb = c_pool.tile([P, nh], F32)
        b2_sb = c_pool.tile([P, nd], F32)
        if fast_io:
            bq = getattr(nc, bias_q or "gpsimd")
            bq.dma_start(out=b1_sb, in_=b1_hbm)
            bq.dma_start(out=b2_sb, in_=b2_hbm)
        else:
            nc.sync.dma_start(out=b1_sb, in_=b1_hbm.rearrange("t p -> p t"))
            nc.sync.dma_start(out=b2_sb, in_=b2_hbm.rearrange("t p -> p t"))

        # Weights, resident in SBUF for the whole kernel. Chunked DMAs so
        # compute can start as soon as the first chunks land.
        w1_t = []
        for ih in range(nh):
            wt = w1_pool.tile([P, kd, P], BF16)
            if fast_io:
                nc.sync.dma_start(
                    out=wt.rearrange("p a b -> p (a b)"), in_=w1_v[:, ih, :])
            else:
                nc.sync.dma_start(out=wt, in_=w1_v[:, :, ih * P:(ih + 1) * P])
            w1_t.append(wt)
        w2_t = []
        for ikh in range(nh):
            wt = w2_pool.tile([P, d], BF16)
            getattr(nc, w2_q).dma_start(out=wt, in_=w2_v[:, ikh, :])
            w2_t.append(wt)

        prev_mm = [None]

        def MM(*args, **kwargs):
            bi = nc.tensor.matmul(*args, **kwargs)
            if chain_pe and prev_mm[0] is not None:
                _add_dep_helper(bi.ins, prev_mm[0].ins, sync=False,
                                reason="pe emission order")
            prev_mm[0] = bi
            return bi


        gelu = act
        xt_pre = {}
        if x_mode == "preload":
            for ib in range(nblk):
                xt_pre[ib] = c_pool.tile([P, kd, tblk], BF16,
                                         name=f"xp{ib}", tag=f"xp{ib}")
                nc.sync.dma_start(
                    out=xt_pre[ib],
                    in_=xt_v[:, :, ib * tblk:(ib + 1) * tblk])
        for ib in [i % nb for i in range(nb * repeats)]:
            tb = blk_sizes[ib]
            tsl = slice(blk_off[ib], blk_off[ib] + tb)
            if x0_parts is not None and ib == 0:
                xts = [x0_parts[ik] for ik in range(kd)]
            elif x_mode == "preload":
                xt = xt_pre[ib]
                xts = [xt[:, ik, :] for ik in range(kd)]
            else:
                xt = x_pool.tile([P, kd, tblk], BF16, name="xt", tag="xt")
                xdma.dma_start(out=xt[:, :, 0:tb], in_=xt_v[:, :, tsl])
                xts = [xt[:, ik, 0:tb] for ik in range(kd)]

            # phase 1: hT[h_tile] = gelu(w1.T @ xT + b1)
            ht = []
            if 1 in phases:
                for ih in range(nh):
                    ps = ps1.tile([P, tblk], F32)
                    for ik in range(kd):
                        bi = MM(
                            ps[:, :tb], w1_t[ih][:, ik, :], xts[ik],
                            start=(ik == 0), stop=(ik == kd - 1),
                        )
                        if x0_manual_parts is not None and ib == 0 and ih == 0:
                            _add_dep_helper(bi.ins, x0_manual_dmas[ik].ins,
                                            sync=True, reason="x0 manual dma")
                    hs = h_pool.tile([P, tblk], BF16, name="hs", tag="hs")
                    if act_mode == "gelu":
                        nc.scalar.activation(hs[:, :tb], ps[:, :tb], gelu,
                                             bias=b1_sb[:, ih:ih + 1])
                    else:
                        nc.vector.tensor_copy(hs[:, :tb], ps[:, :tb])
                    ht.append(hs[:, :tb])
            else:
                # diagnostic: fake hT from xt slices (kd divides nh usage)
                for ih in range(nh):
                    hs = h_pool.tile([P, tblk], BF16, name="hs", tag="hs")
                    nc.vector.tensor_copy(hs[:, :tb], xts[ih % kd])
                    ht.append(hs[:, :tb])

            # phase 2: outT[d_tile] = w2.T @ hT + b2
            if 2 in phases:
                for idt in range(nd):
                    ps = ps2.tile([P, tblk], F32)
                    for ikh in range(nh):
                        MM(
                            ps[:, :tb], w2_t[ikh][:, idt * P:(idt + 1) * P],
                            ht[ikh],
                            start=(ikh == 0), stop=(ikh == nh - 1),
                        )
                    ob = o_pool.tile([P, tblk], F32, name="ob", tag="ob")
                    nc.vector.tensor_scalar_add(ob[:, :tb], ps[:, :tb],
                                                b2_sb[:, idt:idt + 1])
                    getattr(nc, out_q).dma_start(
                        out=out_hbm[idt * P:(idt + 1) * P, tsl],
                        in_=ob[:, :tb]
                    )
            elif 1 in phases:
                # keep outputs observable so phase-1 work isn't dead
                idt = 0
                ob = o_pool.tile([P, tblk], F32, name="ob", tag="ob")
                nc.vector.tensor_copy(ob[:, :tb], ht[ib % nh])
                nc.scalar.dma_start(
                    out=out_hbm[idt * P:(idt + 1) * P, tsl], in_=ob[:, :tb]
                )

    nc.compile()
    return nc


# Config used by kernel() — the tuned fast-IO layout.
DEFAULT_CFG = dict(fast_io=True, x0_chunked=True)

_NC_CACHE = {}


def _get_nc():
    if "nc" not in _NC_CACHE:
        _NC_CACHE["nc"] = build_nc(**DEFAULT_CFG)
    return _NC_CACHE["nc"]


def pack_w1(w1e, d=D, h=H):
    """Host pre-tiling of one expert's w1 [d,h] -> [nh, P, kd*P] bf16 so
    each per-ih DMA reads contiguous 2KB per partition."""
    kd, nh = d // P, h // P
    bf16 = ml_dtypes.bfloat16
    w = np.asarray(w1e, np.float32).astype(bf16)
    return np.ascontiguousarray(
        w.reshape(kd, P, nh, P).transpose(2, 1, 0, 3).reshape(nh, P, kd * P))


def _make_in_maps(x, w1, b1, w2, b2):
    bf16 = ml_dtypes.bfloat16
    nh, nd = H // P, D // P
    in_maps = []
    for e in range(E):
        xe = np.asarray(x[:, e], dtype=np.float32).reshape(T, D)
        in_maps.append({
            "xt": np.ascontiguousarray(xe.T).astype(bf16),
            "w1": pack_w1(w1[e]),
            "w2": np.asarray(w2[e], dtype=np.float32).astype(bf16),
            "b1": np.ascontiguousarray(
                np.asarray(b1[e], np.float32).reshape(nh, P).T),
            "b2": np.ascontiguousarray(
                np.asarray(b2[e], np.float32).reshape(nd, P).T),
        })
    return in_maps


def kernel(x, w1, b1, w2, b2):
    nc = _get_nc()
    in_maps = _make_in_maps(x, w1, b1, w2, b2)
    res = run_bass_kernel_spmd(nc, in_maps, core_ids=list(range(E)))

    out = np.empty((B, E, N, D), dtype=np.float32)
    for e in range(E):
        ot = np.asarray(res.results[e]["outT"])            # [D, T]
        out[:, e] = ot.T.reshape(B, N, D)
    return out

